# revision 4
# baseline (speedup 1.0000x reference)
"""Trainium2 Bass kernel for nn_MHA_75110388072824.

Multi-head attention, B=2, T=2048, D=2048, NH=16 heads (hd=128), fp32 in/out,
causal mask, y = softmax(mask((x Wq^T)(x Wk^T)^T / sqrt(hd))) (x Wv^T) Wo^T.

Sharding over 8 NeuronCores: core = b*4 + hg, b in {0,1} batch,
hg in {0..3} a group of 4 heads (tensor-parallel columns of Wq/Wk/Wv,
rows of Wo).  Each core computes a partial output [T, D] = Z_hg @ Wo_hg^T
in bf16; the host sums the 4 head-group partials per batch in fp32.

v3 vs v2:
  - causal mask applied on the tensor engine: one accumulating matmul
    (identity stationary x constant lower-triangle penalty tile) into the
    128-wide diagonal boundary block of the scores psum.  The DVE leaves
    the S -> exp critical chain entirely, so reciprocal/normalize bursts
    no longer stall the PE through the DVE FIFO.
  - phase A input DMAs spread over four queues (wq:sync, wk:vector,
    wv/wo/masks:gpsimd, x:scalar) so K/V weights land before they gate
    the tensor engine.
  - phase C draws its psum tiles from the same pool as the score strips
    (no psum scope turnover bubble between B and C) and alternates the
    output DMA between the sync and gpsimd queues.
"""
import numpy as np
import ml_dtypes

import concourse.bass as bass
import concourse.mybir as mybir
import concourse.tile as tile
from concourse import bacc
from concourse.bass_utils import run_bass_kernel_spmd

P = 128
T = 2048
D = 2048
NH = 16
HPC = 4            # heads per core
HD = 128
NT = T // P        # 16 t-blocks
NC4 = T // 512     # 4 512-chunks
KT = D // P        # 16 k-tiles over D
SCALE = 1.0 / float(np.sqrt(HD))
MASKVAL = np.float32(-1.0e6)   # pre-scale additive penalty; exp -> 0
F32 = mybir.dt.float32
BF16 = mybir.dt.bfloat16
EXP = mybir.ActivationFunctionType.Exp
MULT = mybir.AluOpType.mult

CFG = dict(xs=2, aps=8, strip=5, sps=4, lps=2, zps=2, lr=4, cev=4)


def _phase_a(tc, nc, xT, wqT, wkT, wvT, qk_h, v_sb):
    # ---------------- Phase A: QKV projections into resident SBUF bf16.
    with (
        tc.tile_pool(name="wqkv", bufs=1) as wpool,
        tc.tile_pool(name="xs", bufs=CFG["xs"]) as xpool,
        tc.tile_pool(name="aps", bufs=CFG["aps"], space="PSUM") as aps,
    ):
        wq_s = wpool.tile([P, KT, 512], BF16, tag="wq")
        wk_s = wpool.tile([P, KT, 512], BF16, tag="wk")
        wv_s = wpool.tile([P, KT, 512], BF16, tag="wv")
        xTr = xT.rearrange("(ko p) t -> p ko t", p=P)
        xs0 = xpool.tile([P, KT, 512], BF16, tag="xs", name="xs0")
        wqTr = wqT.rearrange("(ko p) d -> p ko d", p=P)
        wkTr = wkT.rearrange("(ko p) d -> p ko d", p=P)
        wvTr = wvT.rearrange("(ko p) d -> p ko d", p=P)
        # wq/wk interleave on the sync ring in exactly the order the
        # Q/K-interleaved matmul stream consumes them; x chunks stream on
        # the scalar queue, wv on gpsimd.
        for kc in range(4):
            ksl = slice(4 * kc, 4 * (kc + 1))
            nc.scalar.dma_start(xs0[:, ksl], xTr[:, ksl, 0:512])
            nc.sync.dma_start(wq_s[:, ksl], wqTr[:, ksl])
            nc.sync.dma_start(wk_s[:, ksl], wkTr[:, ksl])
        for kc in range(2):
            ksl = slice(8 * kc, 8 * (kc + 1))
            nc.gpsimd.dma_start(wv_s[:, ksl], wvTr[:, ksl])

        for tci in range(NC4):
            if tci == 0:
                xs = xs0
            else:
                xs = xpool.tile([P, KT, 512], BF16, tag="xs")
                for kc in range(2):
                    ksl = slice(8 * kc, 8 * (kc + 1))
                    nc.scalar.dma_start(
                        xs[:, ksl],
                        xTr[:, ksl, 512 * tci:512 * (tci + 1)])
            # Q and K interleaved at k-chunk granularity (8 live psum
            # groups) so the K matmuls never sit behind the full wq DMA.
            pssq = [aps.tile([P, 512], F32, tag="ps", name=f"psq{i}")
                    for i in range(HPC)]
            pssk = [aps.tile([P, 512], F32, tag="ps", name=f"psk{i}")
                    for i in range(HPC)]
            for kc in range(4):
                for w_s, pss in ((wq_s, pssq), (wk_s, pssk)):
                    for h in range(HPC):
                        for k in range(4 * kc, 4 * kc + 4):
                            nc.tensor.matmul(
                                pss[h][:], w_s[:, k, h * P:(h + 1) * P],
                                xs[:, k, :],
                                start=(k == 0), stop=(k == KT - 1))
            for off, pss in ((0, pssq), (T, pssk)):
                for h in range(HPC):
                    nc.vector.tensor_copy(
                        qk_h[h][:, off + 512 * tci:off + 512 * (tci + 1)],
                        pss[h][:])
            pss = [aps.tile([P, 512], F32, tag="ps", name=f"psv{i}")
                   for i in range(HPC)]
            for kc in range(4):
                for sb in range(4):
                    for k in range(4 * kc, 4 * kc + 4):
                        nc.tensor.matmul(
                            pss[sb][:], xs[:, k, sb * P:(sb + 1) * P],
                            wv_s[:, k, :],
                            start=(k == 0), stop=(k == KT - 1))
            for sb in range(4):
                nc.vector.tensor_copy(v_sb[:, 4 * tci + sb, :], pss[sb][:])


def _phase_bc(tc, nc, neg_lt, ident, ones_sq, qk_h, v_sb, zt_tiles, wo_s,
              out):
    # ---------------- Phase B: attention per head, all operands resident.
    # Diagonal-block causal mask runs on the PE: after the scores matmul,
    # one accumulating matmul adds ident.T @ neg_lt (= -1e6 on the strict
    # lower triangle of [s_local, t_local]) into the 128-wide boundary
    # column block.  exp then reads psum directly — no DVE hop.
    with (
        tc.tile_pool(name="strip", bufs=CFG["strip"]) as spool,
        tc.tile_pool(name="lr", bufs=CFG["lr"]) as lrpool,
        tc.tile_pool(name="cev", bufs=CFG["cev"]) as cev,
        tc.tile_pool(name="sps", bufs=CFG["sps"], space="PSUM") as sps,
        tc.tile_pool(name="lps", bufs=CFG["lps"], space="PSUM") as lps,
        tc.tile_pool(name="zps", bufs=CFG["zps"], space="PSUM") as zps,
    ):
        for h in range(HPC):
            qk = qk_h[h]
            for tc2 in range(4):
                ns = 4 * tc2 + 4
                lsum = lps.tile([P, 512], F32, tag="lsum")
                ztp = zps.tile([P, 512], F32, tag="ztp")
                for si in range(ns):
                    q = si - 4 * tc2
                    t0 = max(0, 128 * q)   # left edge of valid t range
                    sp = sps.tile([P, 512], F32, tag="sp")
                    diag = q >= 0
                    nc.tensor.matmul(
                        sp[:, t0:], qk[:, T + si * P:T + (si + 1) * P],
                        qk[:, 512 * tc2 + t0:512 * (tc2 + 1)],
                        start=True, stop=not diag)
                    if diag:
                        nc.tensor.matmul(
                            sp[:, t0:t0 + P], ident[:], neg_lt[:],
                            start=False, stop=True, skip_group_check=True)
                    strip = spool.tile([P, 512], BF16, tag="strip")
                    nc.scalar.activation(strip[:, t0:], sp[:, t0:], EXP,
                                         bias=0.0, scale=SCALE)
                    nc.tensor.matmul(lsum[:, t0:], ones_sq[:],
                                     strip[:, t0:],
                                     start=(si == 0), stop=(si == ns - 1))
                    nc.tensor.matmul(ztp[:, t0:],
                                     v_sb[:, si, h * P:(h + 1) * P],
                                     strip[:, t0:],
                                     start=(si == 0), stop=(si == ns - 1))
                # 1/l and z normalization; chunked so the DVE occupancy
                # stays in short pieces.
                rlb = lrpool.tile([P, 512], F32, tag="rlb")
                for ch in range(2):
                    csl = slice(256 * ch, 256 * (ch + 1))
                    nc.vector.reciprocal(rlb[:, csl], lsum[:, csl])
                    with nc.allow_low_precision(reason="z normalize bf16"):
                        nc.vector.tensor_tensor(
                            zt_tiles[h][:, 512 * tc2 + 256 * ch:
                                        512 * tc2 + 256 * (ch + 1)],
                            ztp[:, csl], rlb[:, csl], MULT)

        # ---------------- Phase C: output projection from SBUF ZT (bf16).
        # psum tiles come from the same pool as the score strips, so there
        # is no psum scope turnover between B and C.
        for ti in range(NT):
            ev = cev.tile([P, 2048], BF16, tag="cev")
            for oc in range(4):
                ps = sps.tile([P, 512], F32, tag="sp", name=f"cp{ti}_{oc}")
                for h in range(HPC):
                    nc.tensor.matmul(
                        ps[:], zt_tiles[h][:, ti * P:(ti + 1) * P],
                        wo_s[:, h, 512 * oc:512 * (oc + 1)],
                        start=(h == 0), stop=(h == HPC - 1))
                nc.vector.tensor_copy(ev[:, 512 * oc:512 * (oc + 1)], ps[:])
            # last row blocks go on the sync (HWDGE) ring: its completion
            # latency is lower, which shortens the end-of-program drain
            if ti % 2 == 0 and ti < 12:
                nc.gpsimd.dma_start(out[ti * P:(ti + 1) * P, :], ev[:])
            else:
                nc.sync.dma_start(out[ti * P:(ti + 1) * P, :], ev[:])


def build(repeat=1, loop_phase=None, phases="ABC", staggered=False):
    nc = bacc.Bacc("TRN2", target_bir_lowering=False, debug=False)
    xT = nc.dram_tensor("xT", [D, T], BF16, kind="ExternalInput").ap()
    wqT = nc.dram_tensor("wqT", [D, 512], BF16, kind="ExternalInput").ap()
    wkT = nc.dram_tensor("wkT", [D, 512], BF16, kind="ExternalInput").ap()
    wvT = nc.dram_tensor("wvT", [D, 512], BF16, kind="ExternalInput").ap()
    woT = nc.dram_tensor("woT", [512, D], BF16, kind="ExternalInput").ap()
    neg_lt_in = nc.dram_tensor("neg_lt", [P, P], BF16,
                               kind="ExternalInput").ap()
    ident_in = nc.dram_tensor("ident", [P, P], BF16,
                              kind="ExternalInput").ap()
    out = nc.dram_tensor("out", [T, D], BF16, kind="ExternalOutput").ap()

    def emit_all():
        with (
            tc.tile_pool(name="res", bufs=1) as rpool,
            tc.tile_pool(name="const", bufs=1) as cpool,
        ):
            qk_h = [rpool.tile([P, 2 * T], BF16, name=f"qk{h}")
                    for h in range(HPC)]
            v_sb = rpool.tile([P, NT, 512], BF16, name="v_sb")
            zt_tiles = [rpool.tile([P, T], BF16, tag=f"zt{h}", name=f"zt{h}")
                        for h in range(HPC)]
            wo_s = rpool.tile([P, HPC, T], BF16, name="wo_s")
            woTr = woT.rearrange("(ko p) d -> p ko d", p=P)
            neg_lt = cpool.tile([P, P], BF16)
            ident = cpool.tile([P, P], BF16)
            ones_sq = cpool.tile([P, P], BF16)
            nc.vector.memset(ones_sq[:], 1.0)
            warm_src = cpool.tile([P, 512], BF16)
            nc.vector.memset(warm_src[:], 0.0)
            # Warmup matmuls: the PE sits in a DMA wait for the first ~10us
            # of each iteration, which leaves the HAM clock gate cold
            # (1.2 GHz) when real work arrives.  Six dummy matmuls on
            # memset tiles trip the activity window and ramp the pipeline
            # while the first x/weight chunks are still in flight.
            with tc.tile_pool(name="warm", bufs=1, space="PSUM") as wps:
                warm_ps = wps.tile([P, 512], F32)
                for _ in range(6):
                    nc.tensor.matmul(warm_ps[:], ones_sq[:], warm_src[:],
                                     start=True, stop=True)
            if "A" in phases:
                _phase_a(tc, nc, xT, wqT, wkT, wvT, qk_h, v_sb)
            # gpsimd ring order: wv (inside phase A) first, then the
            # later-needed constants and Wo.
            nc.gpsimd.dma_start(neg_lt[:], neg_lt_in)
            nc.gpsimd.dma_start(ident[:], ident_in)
            nc.gpsimd.dma_start(wo_s[:], woTr[:])
            if "B" in phases:
                _phase_bc(tc, nc, neg_lt, ident, ones_sq, qk_h, v_sb,
                          zt_tiles, wo_s, out)

    with tile.TileContext(nc) as tc:
        if repeat == 1 and loop_phase is None:
            emit_all()
        elif loop_phase is None:
            with tc.For_i(0, repeat, 1, staggered_reset=staggered):
                emit_all()
        else:
            raise ValueError("loop_phase no longer supported")
    nc.compile()
    return nc


def make_inputs(x, Wq, Wk, Wv, Wo):
    """Host-side sharding: returns in_maps for cores 0..7 (core = b*4 + hg)."""
    bf = ml_dtypes.bfloat16
    # strict lower triangle of [s_local, t_local] gets the penalty
    neg_lt = np.where(np.arange(P)[None, :] < np.arange(P)[:, None],
                      np.float32(MASKVAL), np.float32(0.0)).astype(bf)
    ident = np.eye(P, dtype=np.float32).astype(bf)
    xTs = [np.ascontiguousarray(x[b].T).astype(bf) for b in range(2)]
    WqT = np.ascontiguousarray(Wq.T).astype(bf)
    WkT = np.ascontiguousarray(Wk.T).astype(bf)
    WvT = np.ascontiguousarray(Wv.T).astype(bf)
    WoT = np.ascontiguousarray(Wo.T).astype(bf)
    in_maps = []
    for core in range(8):
        b, hg = core // 4, core % 4
        sl = slice(hg * 512, (hg + 1) * 512)
        in_maps.append({
            "xT": xTs[b],
            "wqT": np.ascontiguousarray(WqT[:, sl]),
            "wkT": np.ascontiguousarray(WkT[:, sl]),
            "wvT": np.ascontiguousarray(WvT[:, sl]),
            "woT": np.ascontiguousarray(WoT[sl, :]),
            "neg_lt": neg_lt,
            "ident": ident,
        })
    return in_maps


_nc_cache = {}


def kernel(x, Wq, Wk, Wv, Wo):
    x = np.asarray(x, dtype=np.float32)
    Wq = np.asarray(Wq, dtype=np.float32)
    Wk = np.asarray(Wk, dtype=np.float32)
    Wv = np.asarray(Wv, dtype=np.float32)
    Wo = np.asarray(Wo, dtype=np.float32)
    if "nc" not in _nc_cache:
        _nc_cache["nc"] = build()
    nc = _nc_cache["nc"]
    in_maps = make_inputs(x, Wq, Wk, Wv, Wo)
    res = run_bass_kernel_spmd(nc, in_maps, core_ids=list(range(8)))
    B = x.shape[0]
    out = np.zeros((B, T, D), dtype=np.float32)
    for core in range(8):
        b = core // 4
        out[b] += res.results[core]["out"].astype(np.float32)
    return out


# revision 5
# speedup vs baseline: 1.0491x; 1.0491x over previous
"""Trainium2 Bass kernel for nn_MHA_75110388072824.

Multi-head attention, B=2, T=2048, D=2048, NH=16 heads (hd=128), fp32 in/out,
causal mask, y = softmax(mask((x Wq^T)(x Wk^T)^T / sqrt(hd))) (x Wv^T) Wo^T.

Sharding over 8 NeuronCores: core = b*4 + hg, b in {0,1} batch,
hg in {0..3} a group of 4 heads (tensor-parallel columns of Wq/Wk/Wv,
rows of Wo).  Each core computes a partial output [T, D] = Z_hg @ Wo_hg^T
in bf16; the host sums the 4 head-group partials per batch in fp32.

Design notes (all matmul operands bf16 -> hardware fast-weight-load; psum
accumulation fp32; measured rel err vs fp32 reference 5.5e-3):
  Phase A: stream x^T by 512-column chunks (chunk-major DRAM layout for
      >=4KB DMA descriptor runs); weights resident in SBUF, shipped on the
      gpsimd SWDGE ring (~3x the HWDGE per-queue bandwidth here); Q and K
      psum groups interleave at k-chunk granularity (8 live groups) so K
      never waits on the tail of the wq DMA; V uses the x chunk as the
      stationary operand.  Q^T/K^T/V stay fully SBUF-resident (no DRAM
      scratch round-trip).
  Phase B: per head, per 512-wide t-chunk: S^T = K-block-stationary @ Q^T
      (causal blocks only).  The causal mask for diagonal blocks runs on
      the tensor engine: one accumulating matmul ident.T @ neg_lt adds a
      -1e6 penalty on the strict lower triangle of the 128-wide boundary
      block, so exp (ACT, psum fp32 -> bf16 strip) follows the scores
      matmul directly with no DVE hop.  A ones-matmul accumulates the
      softmax denominator; PV matmuls accumulate Z^T; 1/l and the
      normalization run on DVE in 256-column chunks (reciprocal is ~8
      cycles/elem — chunking keeps the DVE FIFO from stalling the PE).
  Phase C: out = Z^T^T @ Wo_hg^T from resident bf16 tiles; psum from the
      same pool as the score strips (no psum scope turnover); row blocks
      stream out over both DMA rings, the last ones in 512-column pieces
      so the end-of-program drain waits only on a small final transfer.
  PE warmup: a few dummy matmuls run during the initial DMA wait so the
      HAM clock gate is warm when real work arrives.

Timing (NTFF device profiles): single-shot ~352us, loop-slope ~348us/iter,
vs 435us single-shot for the fp32r scratch-spilling baseline.
"""
import numpy as np
import ml_dtypes

import concourse.bass as bass
import concourse.mybir as mybir
import concourse.tile as tile
from concourse import bacc
from concourse.bass_utils import run_bass_kernel_spmd

P = 128
T = 2048
D = 2048
NH = 16
HPC = 4            # heads per core
HD = 128
NT = T // P        # 16 t-blocks
NC4 = T // 512     # 4 512-chunks
KT = D // P        # 16 k-tiles over D
SCALE = 1.0 / float(np.sqrt(HD))
MASKVAL = np.float32(-1.0e6)   # pre-scale additive penalty; exp -> 0
F32 = mybir.dt.float32
BF16 = mybir.dt.bfloat16
EXP = mybir.ActivationFunctionType.Exp
MULT = mybir.AluOpType.mult

CFG = dict(xs=2, aps=8, strip=5, sps=4, lps=2, zps=2, lr=4, cev=4)


def _phase_a(tc, nc, xT, wqT, wkT, wvT, qk_h, v_sb):
    # ---------------- Phase A: QKV projections into resident SBUF bf16.
    with (
        tc.tile_pool(name="wqkv", bufs=1) as wpool,
        tc.tile_pool(name="xs", bufs=CFG["xs"]) as xpool,
        tc.tile_pool(name="aps", bufs=CFG["aps"], space="PSUM") as aps,
    ):
        wq_s = wpool.tile([P, KT, 512], BF16, tag="wq")
        wk_s = wpool.tile([P, KT, 512], BF16, tag="wk")
        wv_s = wpool.tile([P, KT, 512], BF16, tag="wv")
        xs0 = xpool.tile([P, KT, 512], BF16, tag="xs", name="xs0")
        wqTr = wqT.rearrange("(ko p) d -> p ko d", p=P)
        wkTr = wkT.rearrange("(ko p) d -> p ko d", p=P)
        wvTr = wvT.rearrange("(ko p) d -> p ko d", p=P)
        # All weights go on the gpsimd SWDGE ring — it sustains ~3x the
        # per-queue bandwidth of the HWDGE rings for these 1KB-run access
        # patterns — interleaved wq/wk in consumption order, wv behind.
        # x chunks split across the two HWDGE rings.
        for kc in range(4):
            ksl = slice(4 * kc, 4 * (kc + 1))
            nc.scalar.dma_start(xs0[:, ksl], xT[0, :, ksl])
            nc.gpsimd.dma_start(wq_s[:, ksl], wqTr[:, ksl])
            nc.gpsimd.dma_start(wk_s[:, ksl], wkTr[:, ksl])
        for kc in range(2):
            ksl = slice(8 * kc, 8 * (kc + 1))
            nc.gpsimd.dma_start(wv_s[:, ksl], wvTr[:, ksl])

        for tci in range(NC4):
            if tci == 0:
                xs = xs0
            else:
                xs = xpool.tile([P, KT, 512], BF16, tag="xs")
                eng = nc.sync if tci % 2 == 1 else nc.scalar
                for kc in range(2):
                    ksl = slice(8 * kc, 8 * (kc + 1))
                    eng.dma_start(xs[:, ksl], xT[tci, :, ksl])
            # Q and K interleaved at k-chunk granularity (8 live psum
            # groups) so the K matmuls never sit behind the full wq DMA.
            pssq = [aps.tile([P, 512], F32, tag="ps", name=f"psq{i}")
                    for i in range(HPC)]
            pssk = [aps.tile([P, 512], F32, tag="ps", name=f"psk{i}")
                    for i in range(HPC)]
            for kc in range(4):
                for w_s, pss in ((wq_s, pssq), (wk_s, pssk)):
                    for h in range(HPC):
                        for k in range(4 * kc, 4 * kc + 4):
                            nc.tensor.matmul(
                                pss[h][:], w_s[:, k, h * P:(h + 1) * P],
                                xs[:, k, :],
                                start=(k == 0), stop=(k == KT - 1))
            for off, pss in ((0, pssq), (T, pssk)):
                for h in range(HPC):
                    nc.vector.tensor_copy(
                        qk_h[h][:, off + 512 * tci:off + 512 * (tci + 1)],
                        pss[h][:])
            pss = [aps.tile([P, 512], F32, tag="ps", name=f"psv{i}")
                   for i in range(HPC)]
            for kc in range(4):
                for sb in range(4):
                    for k in range(4 * kc, 4 * kc + 4):
                        nc.tensor.matmul(
                            pss[sb][:], xs[:, k, sb * P:(sb + 1) * P],
                            wv_s[:, k, :],
                            start=(k == 0), stop=(k == KT - 1))
            for sb in range(4):
                nc.vector.tensor_copy(v_sb[:, 4 * tci + sb, :], pss[sb][:])


def _phase_bc(tc, nc, neg_lt, ident, ones_sq, qk_h, v_sb, zt_tiles, wo_s,
              out):
    # ---------------- Phase B: attention per head, all operands resident.
    # Diagonal-block causal mask runs on the PE: after the scores matmul,
    # one accumulating matmul adds ident.T @ neg_lt (= -1e6 on the strict
    # lower triangle of [s_local, t_local]) into the 128-wide boundary
    # column block.  exp then reads psum directly — no DVE hop.
    with (
        tc.tile_pool(name="strip", bufs=CFG["strip"]) as spool,
        tc.tile_pool(name="lr", bufs=CFG["lr"]) as lrpool,
        tc.tile_pool(name="cev", bufs=CFG["cev"]) as cev,
        tc.tile_pool(name="sps", bufs=CFG["sps"], space="PSUM") as sps,
        tc.tile_pool(name="lps", bufs=CFG["lps"], space="PSUM") as lps,
        tc.tile_pool(name="zps", bufs=CFG["zps"], space="PSUM") as zps,
    ):
        for h in range(HPC):
            qk = qk_h[h]
            for tc2 in range(4):
                ns = 4 * tc2 + 4
                lsum = lps.tile([P, 512], F32, tag="lsum")
                ztp = zps.tile([P, 512], F32, tag="ztp")
                for si in range(ns):
                    q = si - 4 * tc2
                    t0 = max(0, 128 * q)   # left edge of valid t range
                    sp = sps.tile([P, 512], F32, tag="sp")
                    diag = q >= 0
                    nc.tensor.matmul(
                        sp[:, t0:], qk[:, T + si * P:T + (si + 1) * P],
                        qk[:, 512 * tc2 + t0:512 * (tc2 + 1)],
                        start=True, stop=not diag)
                    if diag:
                        nc.tensor.matmul(
                            sp[:, t0:t0 + P], ident[:], neg_lt[:],
                            start=False, stop=True, skip_group_check=True)
                    strip = spool.tile([P, 512], BF16, tag="strip")
                    nc.scalar.activation(strip[:, t0:], sp[:, t0:], EXP,
                                         bias=0.0, scale=SCALE)
                    nc.tensor.matmul(lsum[:, t0:], ones_sq[:],
                                     strip[:, t0:],
                                     start=(si == 0), stop=(si == ns - 1))
                    nc.tensor.matmul(ztp[:, t0:],
                                     v_sb[:, si, h * P:(h + 1) * P],
                                     strip[:, t0:],
                                     start=(si == 0), stop=(si == ns - 1))
                # 1/l and z normalization; chunked so the DVE occupancy
                # stays in short pieces.
                rlb = lrpool.tile([P, 512], F32, tag="rlb")
                for ch in range(2):
                    csl = slice(256 * ch, 256 * (ch + 1))
                    nc.vector.reciprocal(rlb[:, csl], lsum[:, csl])
                    with nc.allow_low_precision(reason="z normalize bf16"):
                        nc.vector.tensor_tensor(
                            zt_tiles[h][:, 512 * tc2 + 256 * ch:
                                        512 * tc2 + 256 * (ch + 1)],
                            ztp[:, csl], rlb[:, csl], MULT)

        # ---------------- Phase C: output projection from SBUF ZT (bf16).
        # psum tiles come from the same pool as the score strips, so there
        # is no psum scope turnover between B and C.
        for ti in range(NT):
            ev = cev.tile([P, 2048], BF16, tag="cev")
            for oc in range(4):
                ps = sps.tile([P, 512], F32, tag="sp", name=f"cp{ti}_{oc}")
                for h in range(HPC):
                    nc.tensor.matmul(
                        ps[:], zt_tiles[h][:, ti * P:(ti + 1) * P],
                        wo_s[:, h, 512 * oc:512 * (oc + 1)],
                        start=(h == 0), stop=(h == HPC - 1))
                nc.vector.tensor_copy(ev[:, 512 * oc:512 * (oc + 1)], ps[:])
                if ti >= NT - 2:
                    # ship the last row blocks per-column-chunk right after
                    # each eviction, so the final transfer (and the drain
                    # waiting on its completion receipt) is small
                    nc.sync.dma_start(
                        out[ti * P:(ti + 1) * P, 512 * oc:512 * (oc + 1)],
                        ev[:, 512 * oc:512 * (oc + 1)])
            # earlier row blocks go out whole, alternating queues
            if ti >= NT - 2:
                pass
            elif ti % 2 == 0:
                nc.gpsimd.dma_start(out[ti * P:(ti + 1) * P, :], ev[:])
            else:
                nc.sync.dma_start(out[ti * P:(ti + 1) * P, :], ev[:])


def build(repeat=1, loop_phase=None, phases="ABC", staggered=False, hint=True):
    nc = bacc.Bacc("TRN2", target_bir_lowering=False, debug=False)
    xT = nc.dram_tensor("xT", [NC4, P, KT, 512], BF16,
                        kind="ExternalInput").ap()
    wqT = nc.dram_tensor("wqT", [D, 512], BF16, kind="ExternalInput").ap()
    wkT = nc.dram_tensor("wkT", [D, 512], BF16, kind="ExternalInput").ap()
    wvT = nc.dram_tensor("wvT", [D, 512], BF16, kind="ExternalInput").ap()
    woT = nc.dram_tensor("woT", [512, D], BF16, kind="ExternalInput").ap()
    neg_lt_in = nc.dram_tensor("neg_lt", [P, P], BF16,
                               kind="ExternalInput").ap()
    ident_in = nc.dram_tensor("ident", [P, P], BF16,
                              kind="ExternalInput").ap()
    out = nc.dram_tensor("out", [T, D], BF16, kind="ExternalOutput").ap()

    def emit_all():
        with (
            tc.tile_pool(name="res", bufs=1) as rpool,
            tc.tile_pool(name="const", bufs=1) as cpool,
        ):
            qk_h = [rpool.tile([P, 2 * T], BF16, name=f"qk{h}")
                    for h in range(HPC)]
            v_sb = rpool.tile([P, NT, 512], BF16, name="v_sb")
            zt_tiles = [rpool.tile([P, T], BF16, tag=f"zt{h}", name=f"zt{h}")
                        for h in range(HPC)]
            wo_s = rpool.tile([P, HPC, T], BF16, name="wo_s")
            woTr = woT.rearrange("(ko p) d -> p ko d", p=P)
            neg_lt = cpool.tile([P, P], BF16)
            ident = cpool.tile([P, P], BF16)
            ones_sq = cpool.tile([P, P], BF16)
            nc.vector.memset(ones_sq[:], 1.0)
            warm_src = cpool.tile([P, 512], BF16)
            nc.vector.memset(warm_src[:], 0.0)
            # Warmup matmuls: the PE sits in a DMA wait for the first ~10us
            # of each iteration, which leaves the HAM clock gate cold
            # (1.2 GHz) when real work arrives.  Six dummy matmuls on
            # memset tiles trip the activity window and ramp the pipeline
            # while the first x/weight chunks are still in flight.
            with tc.tile_pool(name="warm", bufs=1, space="PSUM") as wps:
                warm_ps = wps.tile([P, 512], F32)
                for _ in range(6):
                    nc.tensor.matmul(warm_ps[:], ones_sq[:], warm_src[:],
                                     start=True, stop=True)
            if "A" in phases:
                _phase_a(tc, nc, xT, wqT, wkT, wvT, qk_h, v_sb)
            # gpsimd ring order: wv (inside phase A) first, then the
            # later-needed constants and Wo.
            nc.gpsimd.dma_start(neg_lt[:], neg_lt_in)
            nc.gpsimd.dma_start(ident[:], ident_in)
            nc.gpsimd.dma_start(wo_s[:], woTr[:])
            if "B" in phases:
                _phase_bc(tc, nc, neg_lt, ident, ones_sq, qk_h, v_sb,
                          zt_tiles, wo_s, out)

    with tile.TileContext(nc) as tc:
        if repeat == 1 and loop_phase is None:
            emit_all()
        elif loop_phase is None:
            import concourse.mybir as _mb
            hints = (_mb.EngineType.PE, _mb.EngineType.Activation,
                     _mb.EngineType.DVE, _mb.EngineType.SP,
                     _mb.EngineType.Pool) if hint else ()
            with tc.For_i(0, repeat, 1, staggered_reset=staggered,
                          hint_engines=hints):
                emit_all()
        else:
            raise ValueError("loop_phase no longer supported")
    nc.compile()
    return nc


def make_inputs(x, Wq, Wk, Wv, Wo):
    """Host-side sharding: returns in_maps for cores 0..7 (core = b*4 + hg)."""
    bf = ml_dtypes.bfloat16
    # strict lower triangle of [s_local, t_local] gets the penalty
    neg_lt = np.where(np.arange(P)[None, :] < np.arange(P)[:, None],
                      np.float32(MASKVAL), np.float32(0.0)).astype(bf)
    ident = np.eye(P, dtype=np.float32).astype(bf)
    xTs = []
    for b in range(2):
        xt = x[b].T.reshape(KT, P, NC4, 512).transpose(2, 1, 0, 3)
        xTs.append(np.ascontiguousarray(xt).astype(bf))
    WqT = np.ascontiguousarray(Wq.T).astype(bf)
    WkT = np.ascontiguousarray(Wk.T).astype(bf)
    WvT = np.ascontiguousarray(Wv.T).astype(bf)
    WoT = np.ascontiguousarray(Wo.T).astype(bf)
    in_maps = []
    for core in range(8):
        b, hg = core // 4, core % 4
        sl = slice(hg * 512, (hg + 1) * 512)
        in_maps.append({
            "xT": xTs[b],
            "wqT": np.ascontiguousarray(WqT[:, sl]),
            "wkT": np.ascontiguousarray(WkT[:, sl]),
            "wvT": np.ascontiguousarray(WvT[:, sl]),
            "woT": np.ascontiguousarray(WoT[sl, :]),
            "neg_lt": neg_lt,
            "ident": ident,
        })
    return in_maps


_nc_cache = {}


def kernel(x, Wq, Wk, Wv, Wo):
    x = np.asarray(x, dtype=np.float32)
    Wq = np.asarray(Wq, dtype=np.float32)
    Wk = np.asarray(Wk, dtype=np.float32)
    Wv = np.asarray(Wv, dtype=np.float32)
    Wo = np.asarray(Wo, dtype=np.float32)
    if "nc" not in _nc_cache:
        _nc_cache["nc"] = build()
    nc = _nc_cache["nc"]
    in_maps = make_inputs(x, Wq, Wk, Wv, Wo)
    res = run_bass_kernel_spmd(nc, in_maps, core_ids=list(range(8)))
    B = x.shape[0]
    out = np.zeros((B, T, D), dtype=np.float32)
    for core in range(8):
        b = core // 4
        out[b] += res.results[core]["out"].astype(np.float32)
    return out


# revision 6
# speedup vs baseline: 1.0502x; 1.0011x over previous
"""Trainium2 Bass kernel for nn_MHA_75110388072824.

Multi-head attention, B=2, T=2048, D=2048, NH=16 heads (hd=128), fp32 in/out,
causal mask, y = softmax(mask((x Wq^T)(x Wk^T)^T / sqrt(hd))) (x Wv^T) Wo^T.

Sharding over 8 NeuronCores: core = b*4 + hg, b in {0,1} batch,
hg in {0..3} a group of 4 heads (tensor-parallel columns of Wq/Wk/Wv,
rows of Wo).  Each core computes a partial output [T, D] = Z_hg @ Wo_hg^T
in bf16; the host sums the 4 head-group partials per batch in fp32.

Design notes (all matmul operands bf16 -> hardware fast-weight-load; psum
accumulation fp32; measured rel err vs fp32 reference 5.5e-3):
  Phase A: stream x^T by 512-column chunks (chunk-major DRAM layout for
      >=4KB DMA descriptor runs); weights resident in SBUF, shipped on the
      gpsimd SWDGE ring (~3x the HWDGE per-queue bandwidth here); Q and K
      psum groups interleave at k-chunk granularity (8 live groups) so K
      never waits on the tail of the wq DMA; V uses the x chunk as the
      stationary operand.  Q^T/K^T/V stay fully SBUF-resident (no DRAM
      scratch round-trip).
  Phase B: per head, per 512-wide t-chunk: S^T = K-block-stationary @ Q^T
      (causal blocks only).  The causal mask for diagonal blocks runs on
      the tensor engine: one accumulating matmul ident.T @ neg_lt adds a
      -1e6 penalty on the strict lower triangle of the 128-wide boundary
      block, so exp (ACT, psum fp32 -> bf16 strip) follows the scores
      matmul directly with no DVE hop.  A ones-matmul accumulates the
      softmax denominator; PV matmuls accumulate Z^T; 1/l and the
      normalization run on DVE in 256-column chunks (reciprocal is ~8
      cycles/elem — chunking keeps the DVE FIFO from stalling the PE).
  Phase C: out = Z^T^T @ Wo_hg^T from resident bf16 tiles; psum from the
      same pool as the score strips (no psum scope turnover); row blocks
      stream out over both DMA rings, the last ones in 512-column pieces
      so the end-of-program drain waits only on a small final transfer.
  PE warmup: a few dummy matmuls run during the initial DMA wait so the
      HAM clock gate is warm when real work arrives.

Timing (NTFF device profiles): single-shot ~352us, loop-slope ~348us/iter,
vs 435us single-shot for the fp32r scratch-spilling baseline.
"""
import numpy as np
import ml_dtypes

import concourse.bass as bass
import concourse.mybir as mybir
import concourse.tile as tile
from concourse import bacc
from concourse.bass_utils import run_bass_kernel_spmd

P = 128
T = 2048
D = 2048
NH = 16
HPC = 4            # heads per core
HD = 128
NT = T // P        # 16 t-blocks
NC4 = T // 512     # 4 512-chunks
KT = D // P        # 16 k-tiles over D
SCALE = 1.0 / float(np.sqrt(HD))
MASKVAL = np.float32(-1.0e6)   # pre-scale additive penalty; exp -> 0
F32 = mybir.dt.float32
BF16 = mybir.dt.bfloat16
EXP = mybir.ActivationFunctionType.Exp
MULT = mybir.AluOpType.mult

CFG = dict(xs=2, aps=8, strip=5, sps=4, lps=2, zps=2, lr=4, cev=4)


def _phase_a(tc, nc, xT, wqT, wkT, wvT, qk_h, v_sb):
    # ---------------- Phase A: QKV projections into resident SBUF bf16.
    with (
        tc.tile_pool(name="wqkv", bufs=1) as wpool,
        tc.tile_pool(name="xs", bufs=CFG["xs"]) as xpool,
        tc.tile_pool(name="aps", bufs=CFG["aps"], space="PSUM") as aps,
    ):
        wq_s = wpool.tile([P, KT, 512], BF16, tag="wq")
        wk_s = wpool.tile([P, KT, 512], BF16, tag="wk")
        wv_s = wpool.tile([P, KT, 512], BF16, tag="wv")
        xs0 = xpool.tile([P, KT, 512], BF16, tag="xs", name="xs0")
        wqTr = wqT.rearrange("(ko p) d -> p ko d", p=P)
        wkTr = wkT.rearrange("(ko p) d -> p ko d", p=P)
        wvTr = wvT.rearrange("(ko p) d -> p ko d", p=P)
        # All weights go on the gpsimd SWDGE ring — it sustains ~3x the
        # per-queue bandwidth of the HWDGE rings for these 1KB-run access
        # patterns — interleaved wq/wk in consumption order, wv behind.
        # x chunks split across the two HWDGE rings.
        # leading slices are 2 k-tiles so the very first matmuls gate on
        # ~128KB instead of 512KB
        lead = [(0, 2), (2, 4), (4, 6), (6, 8), (8, 12), (12, 16)]
        for lo, hi in lead:
            ksl = slice(lo, hi)
            nc.scalar.dma_start(xs0[:, ksl], xT[0, :, ksl])
            nc.gpsimd.dma_start(wq_s[:, ksl], wqTr[:, ksl])
            nc.gpsimd.dma_start(wk_s[:, ksl], wkTr[:, ksl])
        for kc in range(2):
            ksl = slice(8 * kc, 8 * (kc + 1))
            nc.gpsimd.dma_start(wv_s[:, ksl], wvTr[:, ksl])

        for tci in range(NC4):
            if tci == 0:
                xs = xs0
            else:
                xs = xpool.tile([P, KT, 512], BF16, tag="xs")
                eng = nc.sync if tci % 2 == 1 else nc.scalar
                for kc in range(2):
                    ksl = slice(8 * kc, 8 * (kc + 1))
                    eng.dma_start(xs[:, ksl], xT[tci, :, ksl])
            # Q and K interleaved at k-chunk granularity (8 live psum
            # groups) so the K matmuls never sit behind the full wq DMA.
            pssq = [aps.tile([P, 512], F32, tag="ps", name=f"psq{i}")
                    for i in range(HPC)]
            pssk = [aps.tile([P, 512], F32, tag="ps", name=f"psk{i}")
                    for i in range(HPC)]
            for kc in range(4):
                for w_s, pss in ((wq_s, pssq), (wk_s, pssk)):
                    for h in range(HPC):
                        for k in range(4 * kc, 4 * kc + 4):
                            nc.tensor.matmul(
                                pss[h][:], w_s[:, k, h * P:(h + 1) * P],
                                xs[:, k, :],
                                start=(k == 0), stop=(k == KT - 1))
            for off, pss in ((0, pssq), (T, pssk)):
                for h in range(HPC):
                    nc.vector.tensor_copy(
                        qk_h[h][:, off + 512 * tci:off + 512 * (tci + 1)],
                        pss[h][:])
            pss = [aps.tile([P, 512], F32, tag="ps", name=f"psv{i}")
                   for i in range(HPC)]
            for kc in range(4):
                for sb in range(4):
                    for k in range(4 * kc, 4 * kc + 4):
                        nc.tensor.matmul(
                            pss[sb][:], xs[:, k, sb * P:(sb + 1) * P],
                            wv_s[:, k, :],
                            start=(k == 0), stop=(k == KT - 1))
            for sb in range(4):
                nc.vector.tensor_copy(v_sb[:, 4 * tci + sb, :], pss[sb][:])


def _phase_bc(tc, nc, neg_lt, ident, ones_sq, qk_h, v_sb, zt_tiles, wo_s,
              out):
    # ---------------- Phase B: attention per head, all operands resident.
    # Diagonal-block causal mask runs on the PE: after the scores matmul,
    # one accumulating matmul adds ident.T @ neg_lt (= -1e6 on the strict
    # lower triangle of [s_local, t_local]) into the 128-wide boundary
    # column block.  exp then reads psum directly — no DVE hop.
    with (
        tc.tile_pool(name="strip", bufs=CFG["strip"]) as spool,
        tc.tile_pool(name="lr", bufs=CFG["lr"]) as lrpool,
        tc.tile_pool(name="cev", bufs=CFG["cev"]) as cev,
        tc.tile_pool(name="sps", bufs=CFG["sps"], space="PSUM") as sps,
        tc.tile_pool(name="lps", bufs=CFG["lps"], space="PSUM") as lps,
        tc.tile_pool(name="zps", bufs=CFG["zps"], space="PSUM") as zps,
    ):
        for h in range(HPC):
            qk = qk_h[h]
            for tc2 in range(4):
                ns = 4 * tc2 + 4
                lsum = lps.tile([P, 512], F32, tag="lsum")
                ztp = zps.tile([P, 512], F32, tag="ztp")
                for si in range(ns):
                    q = si - 4 * tc2
                    t0 = max(0, 128 * q)   # left edge of valid t range
                    sp = sps.tile([P, 512], F32, tag="sp")
                    diag = q >= 0
                    nc.tensor.matmul(
                        sp[:, t0:], qk[:, T + si * P:T + (si + 1) * P],
                        qk[:, 512 * tc2 + t0:512 * (tc2 + 1)],
                        start=True, stop=not diag)
                    if diag:
                        nc.tensor.matmul(
                            sp[:, t0:t0 + P], ident[:], neg_lt[:],
                            start=False, stop=True, skip_group_check=True)
                    strip = spool.tile([P, 512], BF16, tag="strip")
                    nc.scalar.activation(strip[:, t0:], sp[:, t0:], EXP,
                                         bias=0.0, scale=SCALE)
                    nc.tensor.matmul(lsum[:, t0:], ones_sq[:],
                                     strip[:, t0:],
                                     start=(si == 0), stop=(si == ns - 1))
                    nc.tensor.matmul(ztp[:, t0:],
                                     v_sb[:, si, h * P:(h + 1) * P],
                                     strip[:, t0:],
                                     start=(si == 0), stop=(si == ns - 1))
                # 1/l and z normalization; chunked so the DVE occupancy
                # stays in short pieces.
                rlb = lrpool.tile([P, 512], F32, tag="rlb")
                for ch in range(2):
                    csl = slice(256 * ch, 256 * (ch + 1))
                    nc.vector.reciprocal(rlb[:, csl], lsum[:, csl])
                    with nc.allow_low_precision(reason="z normalize bf16"):
                        nc.vector.tensor_tensor(
                            zt_tiles[h][:, 512 * tc2 + 256 * ch:
                                        512 * tc2 + 256 * (ch + 1)],
                            ztp[:, csl], rlb[:, csl], MULT)

        # ---------------- Phase C: output projection from SBUF ZT (bf16).
        # psum tiles come from the same pool as the score strips, so there
        # is no psum scope turnover between B and C.
        for ti in range(NT):
            ev = cev.tile([P, 2048], BF16, tag="cev")
            for oc in range(4):
                ps = sps.tile([P, 512], F32, tag="sp", name=f"cp{ti}_{oc}")
                for h in range(HPC):
                    nc.tensor.matmul(
                        ps[:], zt_tiles[h][:, ti * P:(ti + 1) * P],
                        wo_s[:, h, 512 * oc:512 * (oc + 1)],
                        start=(h == 0), stop=(h == HPC - 1))
                nc.vector.tensor_copy(ev[:, 512 * oc:512 * (oc + 1)], ps[:])
                if ti >= NT - 2:
                    # ship the last row blocks per-column-chunk right after
                    # each eviction, so the final transfer (and the drain
                    # waiting on its completion receipt) is small
                    nc.sync.dma_start(
                        out[ti * P:(ti + 1) * P, 512 * oc:512 * (oc + 1)],
                        ev[:, 512 * oc:512 * (oc + 1)])
            # earlier row blocks go out whole, alternating queues
            if ti >= NT - 2:
                pass
            elif ti % 2 == 0:
                nc.gpsimd.dma_start(out[ti * P:(ti + 1) * P, :], ev[:])
            else:
                nc.sync.dma_start(out[ti * P:(ti + 1) * P, :], ev[:])


def build(repeat=1, loop_phase=None, phases="ABC", staggered=False, hint=True):
    nc = bacc.Bacc("TRN2", target_bir_lowering=False, debug=False)
    xT = nc.dram_tensor("xT", [NC4, P, KT, 512], BF16,
                        kind="ExternalInput").ap()
    wqT = nc.dram_tensor("wqT", [D, 512], BF16, kind="ExternalInput").ap()
    wkT = nc.dram_tensor("wkT", [D, 512], BF16, kind="ExternalInput").ap()
    wvT = nc.dram_tensor("wvT", [D, 512], BF16, kind="ExternalInput").ap()
    woT = nc.dram_tensor("woT", [512, D], BF16, kind="ExternalInput").ap()
    neg_lt_in = nc.dram_tensor("neg_lt", [P, P], BF16,
                               kind="ExternalInput").ap()
    ident_in = nc.dram_tensor("ident", [P, P], BF16,
                              kind="ExternalInput").ap()
    out = nc.dram_tensor("out", [T, D], BF16, kind="ExternalOutput").ap()

    def emit_all():
        with (
            tc.tile_pool(name="res", bufs=1) as rpool,
            tc.tile_pool(name="const", bufs=1) as cpool,
        ):
            qk_h = [rpool.tile([P, 2 * T], BF16, name=f"qk{h}")
                    for h in range(HPC)]
            v_sb = rpool.tile([P, NT, 512], BF16, name="v_sb")
            zt_tiles = [rpool.tile([P, T], BF16, tag=f"zt{h}", name=f"zt{h}")
                        for h in range(HPC)]
            wo_s = rpool.tile([P, HPC, T], BF16, name="wo_s")
            woTr = woT.rearrange("(ko p) d -> p ko d", p=P)
            neg_lt = cpool.tile([P, P], BF16)
            ident = cpool.tile([P, P], BF16)
            ones_sq = cpool.tile([P, P], BF16)
            nc.vector.memset(ones_sq[:], 1.0)
            warm_src = cpool.tile([P, 512], BF16)
            nc.vector.memset(warm_src[:], 0.0)
            # Warmup matmuls: the PE sits in a DMA wait for the first ~10us
            # of each iteration, which leaves the HAM clock gate cold
            # (1.2 GHz) when real work arrives.  Six dummy matmuls on
            # memset tiles trip the activity window and ramp the pipeline
            # while the first x/weight chunks are still in flight.
            with tc.tile_pool(name="warm", bufs=1, space="PSUM") as wps:
                warm_ps = wps.tile([P, 512], F32)
                for _ in range(6):
                    nc.tensor.matmul(warm_ps[:], ones_sq[:], warm_src[:],
                                     start=True, stop=True)
            if "A" in phases:
                _phase_a(tc, nc, xT, wqT, wkT, wvT, qk_h, v_sb)
            # gpsimd ring order: wv (inside phase A) first, then the
            # later-needed constants and Wo.
            nc.gpsimd.dma_start(neg_lt[:], neg_lt_in)
            nc.gpsimd.dma_start(ident[:], ident_in)
            nc.gpsimd.dma_start(wo_s[:], woTr[:])
            if "B" in phases:
                _phase_bc(tc, nc, neg_lt, ident, ones_sq, qk_h, v_sb,
                          zt_tiles, wo_s, out)

    with tile.TileContext(nc) as tc:
        if repeat == 1 and loop_phase is None:
            emit_all()
        elif loop_phase is None:
            import concourse.mybir as _mb
            hints = (_mb.EngineType.PE, _mb.EngineType.Activation,
                     _mb.EngineType.DVE, _mb.EngineType.SP,
                     _mb.EngineType.Pool) if hint else ()
            with tc.For_i(0, repeat, 1, staggered_reset=staggered,
                          hint_engines=hints):
                emit_all()
        else:
            raise ValueError("loop_phase no longer supported")
    nc.compile()
    return nc


def make_inputs(x, Wq, Wk, Wv, Wo):
    """Host-side sharding: returns in_maps for cores 0..7 (core = b*4 + hg)."""
    bf = ml_dtypes.bfloat16
    # strict lower triangle of [s_local, t_local] gets the penalty
    neg_lt = np.where(np.arange(P)[None, :] < np.arange(P)[:, None],
                      np.float32(MASKVAL), np.float32(0.0)).astype(bf)
    ident = np.eye(P, dtype=np.float32).astype(bf)
    xTs = []
    for b in range(2):
        xt = x[b].T.reshape(KT, P, NC4, 512).transpose(2, 1, 0, 3)
        xTs.append(np.ascontiguousarray(xt).astype(bf))
    WqT = np.ascontiguousarray(Wq.T).astype(bf)
    WkT = np.ascontiguousarray(Wk.T).astype(bf)
    WvT = np.ascontiguousarray(Wv.T).astype(bf)
    WoT = np.ascontiguousarray(Wo.T).astype(bf)
    in_maps = []
    for core in range(8):
        b, hg = core // 4, core % 4
        sl = slice(hg * 512, (hg + 1) * 512)
        in_maps.append({
            "xT": xTs[b],
            "wqT": np.ascontiguousarray(WqT[:, sl]),
            "wkT": np.ascontiguousarray(WkT[:, sl]),
            "wvT": np.ascontiguousarray(WvT[:, sl]),
            "woT": np.ascontiguousarray(WoT[sl, :]),
            "neg_lt": neg_lt,
            "ident": ident,
        })
    return in_maps


_nc_cache = {}


def kernel(x, Wq, Wk, Wv, Wo):
    x = np.asarray(x, dtype=np.float32)
    Wq = np.asarray(Wq, dtype=np.float32)
    Wk = np.asarray(Wk, dtype=np.float32)
    Wv = np.asarray(Wv, dtype=np.float32)
    Wo = np.asarray(Wo, dtype=np.float32)
    if "nc" not in _nc_cache:
        _nc_cache["nc"] = build()
    nc = _nc_cache["nc"]
    in_maps = make_inputs(x, Wq, Wk, Wv, Wo)
    res = run_bass_kernel_spmd(nc, in_maps, core_ids=list(range(8)))
    B = x.shape[0]
    out = np.zeros((B, T, D), dtype=np.float32)
    for core in range(8):
        b = core // 4
        out[b] += res.results[core]["out"].astype(np.float32)
    return out
